# revision 15
# baseline (speedup 1.0000x reference)
"""Trainium2 Bass kernel v3 for moe_routing (nn_CITADEL_15118284882566).

Math: per pair b,

    out[b] = sum_q qw[b,q] * max(0, max_{l,kd} sims[b,q,l] * dw[b,l,kd]
                                 * [d_id[b,l,kd] == q_id[b,q]])
             + dot(q_cls[b], d_cls[b])

Device strategy (data-parallel over B across 8 cores, 64 pairs/core,
16 groups of 4 pairs; partitions = 4 pairs x 32 queries):

1. DIFF = (d_id - q_id) + 2^-12 * dw via K-stacked fp16 matmuls
   (ids split hi/lo so every operand is fp16-exact) into ONE 5-bank
   PSUM tile d5 [128, 2560].
2. Abs pass: |2^17 * DIFF| -> f16.  match -> 2^5*dw (finite, exact);
   non-match -> saturates to +inf.  Split across engines: ACT does
   planes 0..3 (2 ops), Pool (gpsimd) does plane 4 via tensor_scalar
   (mult 2^17, abs_max 0).
3. DVE min-tree over the 5 kd planes (batched 2 groups wide) ->
   dmn = 2^5*dw (match) or +inf.
4. NaN fixup in ONE fused op: msk = (dmn*0) + dmn  -> {2^5*dw, NaN}
   via scalar_tensor_tensor.
5. Fused product+reduce per group via tensor_tensor_reduce:
   prd = sims * msk, accum = max-reduce(prd, initial=0.0)  (the
   initial 0 provides the reference's implicit relu; NaN lanes are
   skipped by the hw max ALU).  accum lands in res_raw[:, g].
6. Epilogue: res = res_raw * qw (one batched op), one-hot matmul
   (scaled 2^-5) sums the 32 queries per pair; cls dots via
   elementwise mult + ones matmul.

All input DMAs ride the Sync HWDGE queue so Pool stays free for the
abs offload.  PSUM: d5 5 banks + sims 3 bufs x 1 bank = 8 (cls/tok
spool tiles share the sims pool via tag).
"""
import sys

sys.path.insert(0, "/opt/trn_rl_repo")

import numpy as np

B, LQ, LD, KQ, KD, D = 512, 32, 512, 1, 5, 128
NCORES = 8
BPC = B // NCORES          # 64 pairs per core
NB = 4                     # pairs per group
G = BPC // NB              # 16 groups
P = 128
JD = KD * LD               # 2560
KSTACK = 14
EPS = 2.0 ** -12
RSCALE = 2.0 ** 17         # abs input scale (non-match -> f16 inf)

_CACHED = {}


def _register_citmask():
    """Fused mask+product+max-reduce custom DVE op:
    out[k] = select(dmn[k] < 65504, sims[k]*dmn[k], 0)
    accum  = max(0, max_k out[k])   (init 0 == the reference relu)
    No NaN/inf ever reaches the accumulator: inf lanes are routed to 0."""
    from concourse import dve_ops as DO
    from concourse.dve_spec import Spec, Src0, Src1, C0, Zero, select, lower, AluOp
    from concourse.dve_uop import DveOpSpec
    import numpy as np_

    for o in DO.OPS:
        if o.name == "CITMASK":
            return o

    def _ref(in0, in1, s0, s1, imm2):
        body = np_.where(in1 < s0, in0 * in1, 0.0).astype(np_.float32)
        acc = np_.maximum(
            body.reshape(body.shape[0], -1).max(axis=-1, keepdims=True), 0.0)
        return body, acc

    spec = Spec(body=select(Src1 < C0, Src0 * Src1, Zero),
                accum=AluOp.MAX, accum_init=Zero, reference=_ref)
    op = DO.DveOp("CITMASK", spec, subdim=False, uops_sha={})
    _append_op(DO, op, spec)
    return op


def _append_op(DO, op, spec):
    from concourse.dve_spec import lower
    from concourse.dve_uop import DveOpSpec
    DO.OPS.append(op)
    DO.CUSTOM_DVE_SPECS[op.name] = spec
    DO._SUB_OPCODE_FOR_NAME[op.name] = DO._CUSTOM_DVE_ROW_BASE + len(DO.OPS) - 1
    for ver in ("v3", "v4"):
        uops = lower(spec, ver=ver)
        op.uops_sha[ver] = DveOpSpec(
            name=op.name, opcode=DO.get_dve_sub_opcode(op.name),
            uops=uops, rd1_en=True).sha(ver)


def _register_absmin2():
    """min(|in0*s0|, |in1*s0|) — fused abs+min of two planes (SBUF or one
    PSUM stream; the DVE cannot read two PSUM streams in one op)."""
    from concourse import dve_ops as DO
    from concourse.dve_spec import Spec, Src0, Src1, C0, Zero, maxx, minn
    import numpy as np_

    for o in DO.OPS:
        if o.name == "CITABSMIN2":
            return o

    def _ref(in0, in1, s0, s1, imm2):
        return np_.minimum(np_.abs(in0 * s0), np_.abs(in1 * s0)).astype(np_.float32)

    a = Src0 * C0
    b = Src1 * C0
    spec = Spec(body=minn(maxx(a, Zero - a), maxx(b, Zero - b)), reference=_ref)
    op = DO.DveOp("CITABSMIN2", spec, subdim=False, uops_sha={})
    _append_op(DO, op, spec)
    return op


def _register_absmin1():
    """min(|in0*s0|, in1) — fuses one plane's f32->f16 abs (PSUM stream)
    with a min-tree level against an already-abs'd plane (SBUF)."""
    from concourse import dve_ops as DO
    from concourse.dve_spec import Spec, Src0, Src1, C0, Zero, maxx, minn
    import numpy as np_

    for o in DO.OPS:
        if o.name == "CITABSMIN1":
            return o

    def _ref(in0, in1, s0, s1, imm2):
        return np_.minimum(np_.abs(in0 * s0), in1).astype(np_.float32)

    a = Src0 * C0
    spec = Spec(body=minn(maxx(a, Zero - a), Src1), reference=_ref)
    op = DO.DveOp("CITABSMIN1", spec, subdim=False, uops_sha={})
    _append_op(DO, op, spec)
    return op

DEFAULT_OPTS = dict(
    dma_eng="gpsimd",      # engine for the per-group input DMAs
    io_bufs=4,
    big_bufs=2,
    warmup=0,              # dummy matmuls to hold the PE p-state up
    pool_tree=False,       # illegal: Pool rejects TensorTensor
    use_citmask=True,      # fused select-mask+product+max-reduce custom op
    use_absmin2=True,      # fused abs+min of planes 3,4 on DVE (off ACT)
    sims_bufs=3,
)


def _build_module(**kw):
    import os
    opts = dict(DEFAULT_OPTS)
    for a in os.environ.get("KOPTS", "").split(","):
        if "=" in a:
            k, v = a.split("=")
            opts[k] = ((v == "True") if v in ("True", "False")
                       else (int(v) if v.lstrip("-").isdigit() else v))
    opts.update(kw)
    import concourse.bacc as bacc
    import concourse.mybir as mybir
    from concourse import tile

    f16 = mybir.dt.float16
    f32 = mybir.dt.float32
    Alu = mybir.AluOpType
    Act = mybir.ActivationFunctionType

    _CITMASK = _register_citmask()
    _ABSMIN1 = _register_absmin1()

    nc = bacc.Bacc("TRN2", target_bir_lowering=False, debug=False)

    dq_d = nc.dram_tensor("dq", [G, D, NB * (LD + LQ)], f16, kind="ExternalInput")
    rhx_d = nc.dram_tensor("rhx", [G, KSTACK, JD + P], f16, kind="ExternalInput")
    epi_d = nc.dram_tensor("epi", [P, 2 * BPC + NB + 1 + G], f32, kind="ExternalInput")

    out_d = nc.dram_tensor("out", [NB, G + BPC], f32, kind="ExternalOutput")

    with tile.TileContext(nc) as tc:
        with (
            tc.tile_pool(name="sb_io", bufs=opts["io_bufs"]) as sb_io,
            tc.tile_pool(name="sb_big", bufs=opts["big_bufs"]) as sb_big,
            tc.tile_pool(name="sb_wk", bufs=3) as sb_wk,
            tc.tile_pool(name="sb_res", bufs=1) as sb_res,
            tc.tile_pool(name="ps_a", bufs=1, space="PSUM") as ps_a,
            tc.tile_pool(name="ps_b", bufs=1, space="PSUM") as ps_b,
            tc.tile_pool(name="ps_s", bufs=opts["sims_bufs"], space="PSUM") as ps_s,
        ):
            epi_t = sb_res.tile([P, 2 * BPC + NB + 1 + G], f32)
            nc.sync.dma_start(epi_t[:], epi_d[:])
            qw_all = epi_t[:, 2 * BPC + NB + 1:]

            # cls-dot path depends only on epi: run it up front so the tail
            # is just the tiny tok matmul + one combined output DMA
            qcT_t = epi_t[:, 0:BPC]
            dcT_t = epi_t[:, BPC:2 * BPC]
            e4s_t = epi_t[:, 2 * BPC:2 * BPC + NB]
            ones_t = epi_t[:, 2 * BPC + NB:2 * BPC + NB + 1]
            out_sb = sb_res.tile([NB, G + BPC], f32)
            nc.gpsimd.memset(out_sb[:], 0.0)
            res_raw = sb_res.tile([P, G], f32)
            cp = sb_res.tile([D, BPC], f32)
            nc.vector.tensor_tensor(cp[:], qcT_t, dcT_t, Alu.mult)
            cls_ps = ps_s.tile([1, BPC], f32, name="cls_ps", tag="spool")
            nc.tensor.matmul(cls_ps[:], ones_t, cp[:], start=True, stop=True)
            nc.vector.tensor_copy(out_sb[0:1, G:], cls_ps[:])

            dma = nc.gpsimd if opts["dma_eng"] == "gpsimd" else nc.sync

            if opts["warmup"]:
                # keep the PE busy during the initial DMA fill so the HAM
                # p-state ramp completes before the real matmuls start
                ne = 2 * BPC + NB + 1 + G
                wu = ps_s.tile([P, ne], f32, name="wu", tag="spool")
                for _ in range(opts["warmup"]):
                    nc.tensor.matmul(wu[:], epi_t[:, 0:P], epi_t[:],
                                     start=True, stop=True)

            for g in range(G):
                rhx_t = sb_io.tile([KSTACK, JD + P], f16, name="rhx_t")
                dq_t = sb_io.tile([D, NB * (LD + LQ)], f16, name="dq_t")
                (nc.sync if g == 0 else dma).dma_start(rhx_t[:], rhx_d[g, :, :])
                (nc.sync if g == 0 else dma).dma_start(dq_t[:], dq_d[g, :, :])
                dT_t = dq_t[:, 0:NB * LD]
                qTx_t = dq_t[:, NB * LD:]

                lhsT = rhx_t[:, JD:JD + P]
                dA = ps_a.tile([P, 2 * LD], f32, name="dA")
                dB = ps_b.tile([P, 3 * LD], f32, name="dB")
                # one matmul output <= one PSUM bank (512 fp32 cols)
                for k in range(2):
                    nc.tensor.matmul(dA[:, k * LD:(k + 1) * LD], lhsT,
                                     rhx_t[:, k * LD:(k + 1) * LD],
                                     start=True, stop=True)
                for k in range(3):
                    nc.tensor.matmul(dB[:, k * LD:(k + 1) * LD], lhsT,
                                     rhx_t[:, (2 + k) * LD:(3 + k) * LD],
                                     start=True, stop=True)

                s_ps = ps_s.tile([P, LD], f32, name="s_ps", tag="spool")
                for b in range(NB):
                    nc.tensor.matmul(
                        s_ps[b * LQ:(b + 1) * LQ, :],
                        qTx_t[:, b * LQ:(b + 1) * LQ],
                        dT_t[:, b * LD:(b + 1) * LD],
                        start=True, stop=True,
                        tile_position=(0, b * LQ),
                    )

                # ---- abs pass: ACT does planes {0,1} and {2,4};
                # DVE ABSMIN1 fuses plane 3's abs (PSUM) + min vs plane 4
                half = g % 2
                if half == 0:
                    rA = sb_big.tile([P, 2, 2, LD], f16, name="rA")
                    rB = sb_big.tile([P, 2, 3, LD], f16, name="rB")
                    m34 = sb_big.tile([P, 2, LD], f16, name="m34")
                    sps_prev = s_ps
                rAf = rA.rearrange("p u v c -> p (u v c)")
                rBf = rB.rearrange("p u v c -> p (u v c)")
                nc.scalar.activation(rAf[:, half * 2 * LD:(half + 1) * 2 * LD],
                                     dA[:], Act.Abs, bias=0.0, scale=RSCALE)
                if opts["use_absmin2"]:
                    dB3 = dB.rearrange("p (k c) -> p k c", k=3)
                    rBh = rB[:, half, :, :]
                    # one strided ACT op for planes 2 and 4 -> rB[half,{0,1}]
                    nc.scalar.activation(rBh[:, 0:2, :], dB3[:, 0::2, :],
                                         Act.Abs, bias=0.0, scale=RSCALE)
                    nc.vector._custom_dve(
                        _ABSMIN1, out=m34[:, half, :],
                        in0=dB[:, LD:2 * LD], in1=rBh[:, 1, :],
                        s0=RSCALE)
                else:
                    nc.scalar.activation(rBf[:, half * 3 * LD:(half + 1) * 3 * LD],
                                         dB[:], Act.Abs, bias=0.0, scale=RSCALE)
                if half == 0:
                    continue

                # ---- min-tree batched across the 2 groups
                t01 = sb_wk.tile([P, 2, LD], f16, name="t01")
                dmn = sb_wk.tile([P, 2, LD], f16, name="dmn")
                nc.vector.tensor_tensor(t01[:], rA[:, :, 0, :], rA[:, :, 1, :],
                                        Alu.min)
                if opts["use_absmin2"]:
                    nc.vector.tensor_tensor(t01[:], t01[:], rB[:, :, 0, :],
                                            Alu.min)
                    nc.vector.tensor_tensor(dmn[:], t01[:], m34[:], Alu.min)
                else:
                    t23 = sb_wk.tile([P, 2, LD], f16, name="t23")
                    nc.vector.tensor_tensor(t23[:], rB[:, :, 0, :],
                                            rB[:, :, 1, :], Alu.min)
                    nc.vector.tensor_tensor(t01[:], t01[:], t23[:], Alu.min)
                    nc.vector.tensor_tensor(dmn[:], t01[:], rB[:, :, 2, :],
                                            Alu.min)

                if opts["use_citmask"]:
                    dum = sb_wk.tile([P, 1], f32, name="dum")
                    nc.vector._custom_dve(
                        _CITMASK, out=dum.broadcast_to(sps_prev.shape),
                        in0=sps_prev[:], in1=dmn[:, 0, :], s0=65504.0,
                        accum_out=res_raw[:, g - 1:g])
                    nc.vector._custom_dve(
                        _CITMASK, out=dum.broadcast_to(s_ps.shape),
                        in0=s_ps[:], in1=dmn[:, 1, :], s0=65504.0,
                        accum_out=res_raw[:, g:g + 1])
                else:
                    dmnf = dmn.rearrange("p u c -> p (u c)")
                    msk = sb_wk.tile([P, 2 * LD], f16, name="msk")
                    nz = sb_wk.tile([P, 2 * LD], f16, name="nz")
                    nc.vector.tensor_scalar(nz[:], dmnf[:], 0.0, None, Alu.mult)
                    nc.vector.tensor_tensor(msk[:], nz[:], dmnf[:], Alu.add)
                    prd = sb_wk.tile([P, 2, LD], f16, name="prd")
                    nc.vector.tensor_tensor(prd[:, 0, :], sps_prev[:],
                                            msk[:, 0:LD], Alu.mult)
                    nc.vector.tensor_tensor(prd[:, 1, :], s_ps[:],
                                            msk[:, LD:], Alu.mult)
                    mx2 = sb_wk.tile([P, 2], f16, name="mx2")
                    nc.vector.reduce_max(mx2[:], prd[:],
                                         axis=mybir.AxisListType.X)
                    # (mx2 max 0): the max ALU skips NaN, so all-NaN rows
                    # (no match anywhere) land at 0 == the reference value
                    nc.vector.tensor_scalar(res_raw[:, g - 1:g + 1], mx2[:],
                                            0.0, None, Alu.max)

            # ---- epilogue: qw, tok colsums + combined output DMA ----
            resw = sb_res.tile([P, G], f32)
            nc.vector.tensor_tensor(resw[:], res_raw[:], qw_all, Alu.mult)
            tok_ps = ps_s.tile([NB, G], f32, name="tok_ps", tag="spool")
            nc.tensor.matmul(tok_ps[:], e4s_t, resw[:], start=True, stop=True)
            nc.vector.tensor_copy(out_sb[:, 0:G], tok_ps[:])
            nc.sync.dma_start(out_d[:], out_sb[:])

    nc.compile()
    return nc


def _prep_core_inputs(c, q_repr, q_w, q_ids, q_cls, d_repr, d_w, d_ids, d_cls):
    """Pure layout/packing for one core's 64 pairs."""
    s = slice(c * BPC, (c + 1) * BPC)
    qr = q_repr[s]          # [64, 32, 128] f32
    qw = q_w[s, :, 0]       # [64, 32]
    qi = q_ids[s, :, 0]     # [64, 32] int64
    qc = q_cls[s]           # [64, 128]
    dr = d_repr[s]          # [64, 512, 128]
    dw = d_w[s]             # [64, 512, 5]
    di = d_ids[s]           # [64, 512, 5]
    dc = d_cls[s]           # [64, 128]

    dT = np.ascontiguousarray(
        dr.reshape(G, NB, LD, D).transpose(0, 3, 1, 2).reshape(G, D, NB * LD)
    ).astype(np.float16)

    qTx = np.ascontiguousarray(
        qr.reshape(G, NB, LQ, D).transpose(0, 3, 1, 2).reshape(G, D, NB * LQ)
    ).astype(np.float16)
    qww = qw.reshape(G, NB * LQ)  # partition p = 32*b + q

    q_hi = (qi >> 8).astype(np.float32)
    q_lo = (qi & 255).astype(np.float32)
    d_hi = (di >> 8).astype(np.float32)
    d_lo = (di & 255).astype(np.float32)
    dw16 = dw.astype(np.float16).astype(np.float32)

    E = np.zeros((NB, P), np.float32)
    for b in range(NB):
        E[b, b * LQ:(b + 1) * LQ] = 1.0

    # rhx: [G, KSTACK, JD + P]: cols [0, JD) = rhs (kd-major), [JD, JD+P) = lhsT
    rhx = np.zeros((G, KSTACK, JD + P), np.float32)
    rhx[:, 0:4, :JD] = d_hi.reshape(G, NB, LD, KD).transpose(0, 1, 3, 2).reshape(G, NB, JD)
    rhx[:, 4:8, :JD] = d_lo.reshape(G, NB, LD, KD).transpose(0, 1, 3, 2).reshape(G, NB, JD)
    rhx[:, 8, :JD] = 256.0
    rhx[:, 9, :JD] = 1.0
    rhx[:, 10:14, :JD] = dw16.reshape(G, NB, LD, KD).transpose(0, 1, 3, 2).reshape(G, NB, JD)
    rhx[:, 0:4, JD:] = 256.0 * E
    rhx[:, 4:8, JD:] = E
    rhx[:, 8, JD:] = -q_hi.reshape(G, P)
    rhx[:, 9, JD:] = -q_lo.reshape(G, P)
    rhx[:, 10:14, JD:] = EPS * E

    epi = np.zeros((P, 2 * BPC + NB + 1 + G), np.float32)
    epi[:, 0:BPC] = qc.T
    epi[:, BPC:2 * BPC] = dc.T
    for b in range(NB):
        # undo the 2^5 the Abs-scale leaves on matched weights
        epi[b * LQ:(b + 1) * LQ, 2 * BPC + b] = 2.0 ** -5
    epi[:, 2 * BPC + NB] = 1.0
    epi[:, 2 * BPC + NB + 1:] = qww.T

    dq = np.concatenate([dT, qTx], axis=2)  # [G, D, NB*(LD+LQ)]
    return {
        "dq": dq,
        "rhx": rhx.astype(np.float16),
        "epi": epi,
    }


def kernel(q_expert_repr, q_expert_weights, q_expert_ids, q_cls_repr,
           d_expert_repr, d_expert_weights, d_expert_ids, d_cls_repr):
    from concourse.bass_utils import run_bass_kernel_spmd

    q_repr = np.asarray(q_expert_repr, np.float32)
    q_w = np.asarray(q_expert_weights, np.float32)
    q_ids = np.asarray(q_expert_ids, np.int64)
    q_cls = np.asarray(q_cls_repr, np.float32)
    d_repr = np.asarray(d_expert_repr, np.float32)
    d_w = np.asarray(d_expert_weights, np.float32)
    d_ids = np.asarray(d_expert_ids, np.int64)
    d_cls = np.asarray(d_cls_repr, np.float32)

    if "nc" not in _CACHED:
        _CACHED["nc"] = _build_module()
    nc = _CACHED["nc"]

    in_maps = [
        _prep_core_inputs(c, q_repr, q_w, q_ids, q_cls, d_repr, d_w, d_ids, d_cls)
        for c in range(NCORES)
    ]
    rr = run_bass_kernel_spmd(nc, in_maps, core_ids=list(range(NCORES)))

    out = np.zeros((B,), np.float32)
    for c in range(NCORES):
        r = rr.results[c]["out"]            # [NB, G + BPC]
        tok = r[:, 0:G]                     # [NB, G]
        cls = r[0, G:]                      # [BPC]
        out[c * BPC:(c + 1) * BPC] = tok.T.reshape(-1) + cls
    return out
